# revision 2
# baseline (speedup 1.0000x reference)
"""FFTConvNet TRN2 kernel: low-pass filter (cropped matmul-FFT) + 3x3 circular
conv (channel mix) + bias, data-parallel over batch across 8 NeuronCores.

Math: out[b,o] = sum_i lowpass(x[(b+8)%16, i]) (*) w[(o+32)%64, i] + bias[o]
where (*) is 3x3 circular convolution. The batch/channel rolls come from the
reference's fftshift over ALL axes (batch & channel rolls; the input-channel
roll cancels inside the einsum contraction).

Lowpass per image: shifted spectrum is cropped to the 61x61 box that contains
the radius-30 disk; forward = two matmul stages against cropped DFT matrices,
mask applied during PSUM evacuation, inverse = two matmul stages.
"""
import numpy as np
from concourse import bacc, tile, mybir
from concourse.bass_utils import run_bass_kernel_spmd

H = W = 128
NF = 61  # shifted freqs 34..94  <->  band -30..30
NCORE = 8
BPC = 2  # batches per core
CIN = COUT = 64

_CACHE = {}


def _consts():
    r = np.arange(NF)[:, None] - 30.0
    n = np.arange(H)[None, :].astype(np.float64)
    Fc = np.exp(-2j * np.pi * r * n / H)  # [61, 128] cropped shifted DFT
    IFc = (
        np.exp(
            +2j
            * np.pi
            * np.arange(H)[:, None]
            * (np.arange(NF)[None, :] - 30.0)
            / H
        )
        / H
    )  # [128, 61] cropped inverse

    FHpk = np.concatenate([Fc.real.T, Fc.imag.T], axis=1)  # [128, 122]

    rr, cc = np.meshgrid(np.arange(NF), np.arange(NF), indexing="ij")
    Mbox = (((rr - 30) ** 2 + (cc - 30) ** 2) <= 900).astype(np.float64)
    mask4 = np.block([[Mbox, Mbox], [Mbox, Mbox]])  # [122, 122]

    IFhrT, IFhiT = IFc.real.T, IFc.imag.T  # [61, 128]
    IFHA = np.block([[IFhrT, IFhiT], [-IFhiT, IFhrT]])  # [122, 256]
    IFHB = np.block([[-IFhiT, IFhrT], [-IFhrT, -IFhiT]])  # [122, 256]
    IFWr = IFhrT  # [61, 128]
    IFWni = -IFhiT  # [61, 128]
    return FHpk, mask4, IFHA, IFHB, IFWr, IFWni


def _build(nc):
    dt = mybir.dt
    AF = mybir.ActivationFunctionType

    xd = nc.dram_tensor("x", [BPC, CIN, H, W], dt.float32, kind="ExternalInput").ap()
    od = nc.dram_tensor("out", [BPC, COUT, H, W], dt.float32, kind="ExternalOutput").ap()
    fh = nc.dram_tensor("FHpk", [128, 122], dt.float32, kind="ExternalInput").ap()
    gw = nc.dram_tensor("GWpk", [128, 122], dt.float16, kind="ExternalInput").ap()
    m4 = nc.dram_tensor("mask4", [122, 122], dt.float32, kind="ExternalInput").ap()
    iha = nc.dram_tensor("IFHA", [122, 256], dt.float16, kind="ExternalInput").ap()
    ihb = nc.dram_tensor("IFHB", [122, 256], dt.float16, kind="ExternalInput").ap()
    iwr = nc.dram_tensor("IFWr", [NF, 128], dt.float16, kind="ExternalInput").ap()
    iwn = nc.dram_tensor("IFWni", [NF, 128], dt.float16, kind="ExternalInput").ap()
    w9 = nc.dram_tensor("w9", [CIN, 9, COUT], dt.float16, kind="ExternalInput").ap()
    bv = nc.dram_tensor("biasv", [COUT, 1], dt.float32, kind="ExternalInput").ap()

    with tile.TileContext(nc) as tc:
        with (
            tc.tile_pool(name="const", bufs=1) as cp,
            tc.tile_pool(name="work", bufs=3) as wp,
            tc.tile_pool(name="slab", bufs=2) as sp,
            tc.tile_pool(name="pY", bufs=2, space="PSUM") as ppY,
            tc.tile_pool(name="pP2", bufs=2, space="PSUM") as ppP2,
            tc.tile_pool(name="pV", bufs=1, space="PSUM") as ppV,
            tc.tile_pool(name="pXL", bufs=1, space="PSUM") as ppXL,
            tc.tile_pool(name="pC", bufs=2, space="PSUM") as ppC,
        ):
            t_fh = cp.tile([128, 122], dt.float32)
            nc.sync.dma_start(t_fh[:], fh)
            t_gw = cp.tile([128, 122], dt.float16)
            nc.sync.dma_start(t_gw[:], gw)
            t_m4 = cp.tile([122, 122], dt.float32)
            nc.sync.dma_start(t_m4[:], m4)
            t_iha = cp.tile([122, 256], dt.float16)
            nc.sync.dma_start(t_iha[:], iha)
            t_ihb = cp.tile([122, 256], dt.float16)
            nc.sync.dma_start(t_ihb[:], ihb)
            t_iwr = cp.tile([NF, 128], dt.float16)
            nc.sync.dma_start(t_iwr[:], iwr)
            t_iwn = cp.tile([NF, 128], dt.float16)
            nc.sync.dma_start(t_iwn[:], iwn)
            t_w9 = cp.tile([CIN, 9, COUT], dt.float16)
            nc.sync.dma_start(t_w9[:], w9)
            t_bv = cp.tile([COUT, 1], dt.float32)
            nc.sync.dma_start(t_bv[:], bv)

            for b in range(BPC):
                slab = sp.tile([CIN, 131, 131], dt.float16, tag="slab")
                for i in range(CIN):
                    ximg = wp.tile([128, 128], dt.float32, tag="ximg")
                    nc.sync.dma_start(ximg[:], xd[b, i])

                    # S1 (fp32): Y[w, hf-stack] = x.T @ FHpk
                    pY = ppY.tile([128, 122], dt.float32, tag="pY")
                    nc.tensor.matmul(pY[:], ximg[:], t_fh[:], start=True, stop=True)
                    sY = wp.tile([128, 122], dt.float16, tag="sY")
                    nc.vector.tensor_copy(sY[:], pY[:])

                    # S2: P2[hf-stack, wf-stack] = Y.T @ GWpk ; mask on evac
                    pP2 = ppP2.tile([122, 122], dt.float32, tag="pP2")
                    nc.tensor.matmul(pP2[:], sY[:], t_gw[:], start=True, stop=True)
                    sP2 = wp.tile([122, 122], dt.float16, tag="sP2")
                    nc.vector.tensor_mul(sP2[:], pP2[:], t_m4[:])

                    # S3 (invH): V[wf, h-stack] = P2L.T @ IFHA + P2R.T @ IFHB
                    pV = ppV.tile([NF, 256], dt.float32, tag="pV")
                    nc.tensor.matmul(pV[:], sP2[:, 0:NF], t_iha[:], start=True, stop=False)
                    nc.tensor.matmul(pV[:], sP2[:, NF:122], t_ihb[:], start=False, stop=True)
                    sV = wp.tile([NF, 256], dt.float16, tag="sV")
                    nc.scalar.activation(sV[:], pV[:], AF.Identity)

                    # S4 (invW): xl[h, w] = Vr.T @ IFWr + Vi.T @ IFWni
                    pXL = ppXL.tile([128, 128], dt.float32, tag="pXL")
                    nc.tensor.matmul(pXL[:], sV[:, 0:128], t_iwr[:], start=True, stop=False)
                    nc.tensor.matmul(pXL[:], sV[:, 128:256], t_iwn[:], start=False, stop=True)
                    sXL = wp.tile([128, 128], dt.float16, tag="sXL")
                    nc.vector.tensor_copy(sXL[:], pXL[:])

                    # bridge into channel-layout slab (padded for circular conv)
                    nc.sync.dma_start(slab[i : i + 1, 2:130, 2:130], sXL[:])

                # circular pads: cols then rows
                nc.sync.dma_start(slab[:, 2:130, 0:2], slab[:, 2:130, 128:130])
                nc.sync.dma_start(slab[:, 0:2, :], slab[:, 128:130, :])

                # conv 3x3 + bias
                for r0 in range(0, 128, 4):
                    pC = ppC.tile([COUT, 512], dt.float32, tag="pC")
                    k = 0
                    for p in range(3):
                        for q in range(3):
                            nc.tensor.matmul(
                                pC[:],
                                t_w9[:, p * 3 + q, :],
                                slab[:, 2 + r0 - p : 6 + r0 - p, 2 - q : 130 - q],
                                start=(k == 0),
                                stop=(k == 8),
                            )
                            k += 1
                    yb = wp.tile([COUT, 512], dt.float32, tag="yb")
                    nc.scalar.activation(yb[:], pC[:], AF.Identity, bias=t_bv[:, 0:1])
                    ohw = od[b].rearrange("o h w -> o (h w)")
                    nc.sync.dma_start(ohw[:, r0 * 128 : (r0 + 4) * 128], yb[:])


def _get_compiled():
    if "nc" not in _CACHE:
        nc = bacc.Bacc("TRN2", target_bir_lowering=False, debug=False, num_devices=NCORE)
        _build(nc)
        nc.compile()
        _CACHE["nc"] = nc
    return _CACHE["nc"]


def kernel(x, weight, bias):
    x = np.asarray(x, dtype=np.float32)
    weight = np.asarray(weight, dtype=np.float32)
    bias = np.asarray(bias, dtype=np.float32)
    B = x.shape[0]

    nc = _get_compiled()
    FHpk, mask4, IFHA, IFHB, IFWr, IFWni = _consts()

    wdev = np.roll(weight, -32, axis=0)  # out-channel roll
    w9 = np.transpose(wdev, (1, 2, 3, 0)).reshape(CIN, 9, COUT)

    consts = {
        "FHpk": FHpk.astype(np.float32),
        "GWpk": FHpk.astype(np.float16),
        "mask4": mask4.astype(np.float32),
        "IFHA": IFHA.astype(np.float16),
        "IFHB": IFHB.astype(np.float16),
        "IFWr": IFWr.astype(np.float16),
        "IFWni": IFWni.astype(np.float16),
        "w9": w9.astype(np.float16),
        "biasv": bias.reshape(COUT, 1).astype(np.float32),
    }
    in_maps = []
    for c in range(NCORE):
        bs = [(BPC * c + 8) % B, (BPC * c + 9) % B]
        in_maps.append({"x": np.ascontiguousarray(x[bs]), **consts})

    res = run_bass_kernel_spmd(nc, in_maps, list(range(NCORE))).results
    out = np.concatenate([r["out"] for r in res], axis=0)
    return np.ascontiguousarray(out.astype(np.float32))
